# revision 26
# baseline (speedup 1.0000x reference)
"""Trainium2 Bass kernel for nn_EuESN_maml: assemble the 3N x 3N wave-equation
transition matrix A (N = 48*48) from c/dkx/dky fields.

A is all zeros except ~11 diagonals spread over the 9 N x N blocks, two of
which (A12/A21) are structurally zero. The kernel is a DMA memset of the
nonzero block bands plus strided diagonal scatter writes; the run is pure
HBM-write-bound (the fill runs at the ~358 GB/s per-core cap), so
  - the shard is stored as bfloat16 and widened to f32 on the host while
    gathering (<=2^-9 relative error, far inside the 2e-2 gate), and
  - each block row stores only its structurally nonzero band: block row 0
    is full width, block rows 1/2 are rotated so their diagonals land in
    stored columns [0, 2N) and the A12/A21 zero blocks are never written
    (the host zeros canvas provides them on unshard) - 22% fewer bytes.

Sharding (SPMD, 8 cores): block-row index partitioned. Core k owns rows
[288k, 288k+288) of each of the three N-row block rows of A. Per-band
column rotations make diagonal positions identical on every core (single
SPMD program); the host un-rotates with slice copies while gathering.

Engine plan per core:
  vector: small zero-tile memset, then the diagonal value vectors computed
         on 96 partitions ([96,3] row-major packing of each 288-vector)
         with the x/y chains interleaved to hide the no-scoreboard
         semaphore latency; the global max(c) finishes through one 32x32
         DVE block transpose; adjacent-diagonal pairs are written stride-2
         interleaved so one DMA covers both diagonals
  gpsimd: the big zero tile, then the constructed-tile memset
  sync  (HWDGE ring 0): contiguous zero-fill DMAs, all sourcing 128
         partitions, then the constructed tile: the whole block row 2
         band ([288, 4608] stored flat as [128, 10368]) with its three
         diagonals baked in, so nothing scatters after the fill ends
  scalar(HWDGE ring 1): input load, inv-dx broadcast (zero-stride
         replicated read), 2 constructed-tile strip writes, and 7
         diagonal scatter DMAs issued as soon as the zero pieces covering
         their rows have landed (fully inside the fill window)
"""

import math
import sys

import numpy as np

sys.path.insert(0, "/opt/trn_rl_repo")

import concourse.bass as bass
import concourse.mybir as mybir
from concourse.bass_utils import run_bass_kernel_spmd

# ---- problem constants (hardcoded from the nn_EuESN_maml spec) ----
n = 48
N = n * n            # 2304
M3 = 3 * N           # 6912 (output is M3 x M3)
NCORES = 8
B = N // NCORES      # 288 rows per sub-band
DT, CN, KP = 1.0, 0.1, 1e-4
MI = 1.0 / (1.0 / DT - KP / 2.0)          # 1/diagM (diagM is constant)
K0 = (1.0 / DT + KP / 2.0) * MI           # A00 diagonal value (constant)
DXC = (DT / CN) * math.sqrt(2.0)          # dx = DXC * max(c)

WB = 2 * N           # stored width of block rows 1/2 (4608)
CTR = 126            # constructed-tile rows: block-row-2 rows 162..287
J0 = B - CTR         # first sub-band-2-local row handled by the tile (162)

# value layout in vv [96, VW] bf16, value r of a slot at [r//3, base+r%3]:
# singles at 3*s; pairs P0/P1 stride-2 interleaved (two adjacent diagonals
# per 2-element descriptor)
S0, S1, S2, S3, S4, S5, S6 = 0, 3, 6, 9, 12, 15, 18
P0, P1 = 21, 27      # 6 cols each
S7, S8 = 33, 36      # plain copies of P1 halves for the strip sources
VW = 39

NVOPS = 33
# packed input: 8 value vectors as [96, 3] at cols 0..23, then the c grid
# as [32, 72] at cols 24..95 (32 partitions so one 32x32 block transpose
# finishes the global max)
VCOL = 0
CCOL = 24
PKW = CCOL + 72


def _build_program() -> bass.Bass:
    nc = bass.Bass()
    f32 = mybir.dt.float32
    bf16 = mybir.dt.bfloat16

    pk = nc.declare_dram_parameter("pk", [128, PKW], f32, isOutput=False)
    outA = nc.declare_dram_parameter("outA", [B, M3], bf16, isOutput=True)
    outB = nc.declare_dram_parameter("outB", [2 * B, WB], bf16, isOutput=True)

    from contextlib import ExitStack

    with ExitStack() as ctx:
        ec = ctx.enter_context
        zts = ec(nc.sbuf_tensor([128, 864], bf16))     # small zero tile
        ztb = ec(nc.sbuf_tensor([128, M3], bf16))      # big zero tile
        ct = ec(nc.sbuf_tensor([128, WB], bf16))       # constructed tile
        pkb = ec(nc.sbuf_tensor([128, PKW], f32))      # packed inputs
        vv = ec(nc.sbuf_tensor([96, VW], bf16))        # diag value slots
        tmp = ec(nc.sbuf_tensor([96, 36], f32))        # [96,3] scratch x12
        tsq = ec(nc.sbuf_tensor([32, 32], f32))        # transpose in
        tsqT = ec(nc.sbuf_tensor([32, 32], f32))       # transpose out
        rowA = ec(nc.sbuf_tensor([1, 96], f32))        # inv-dx replicated
        ib = ec(nc.sbuf_tensor([96, 1], f32))          # inv-dx per partition
        scal = ec(nc.sbuf_tensor([1, 8], f32))         # scalars
        in_sem = ec(nc.semaphore("in_sem"))
        vchain = ec(nc.semaphore("vchain"))
        gz = ec(nc.semaphore("gz"))
        ctm = ec(nc.semaphore("ctm"))
        bsem = ec(nc.semaphore("bsem"))
        zsemA = ec(nc.semaphore("zsemA"))
        zsemB = ec(nc.semaphore("zsemB"))
        zsemC = ec(nc.semaphore("zsemC"))
        strip_sem = ec(nc.semaphore("strip_sem"))
        ddma = ec(nc.semaphore("ddma"))
        ctd = ec(nc.semaphore("ctd"))
        block = ec(nc.Block())

        def vsl(base, p0=0, np_=96, w=3):
            return vv[p0 : p0 + np_, base : base + w]

        def vstr(base, off):
            # stride-2 interleaved DVE write target inside a pair region
            return bass.AP(vv, base + off, [[VW, 96], [2, 3], [1, 1]])

        def vpair_src(base, np_=96):
            # pair DMA source: 2-element contiguous runs per value index
            return bass.AP(vv, base, [[VW, np_], [2, 3], [1, 2]])

        def tslot(s):
            return tmp[0:96, s * 3 : (s + 1) * 3]

        def sc(i):
            return scal[0:1, i : i + 1]

        def pvec(i):
            return pkb[0:96, VCOL + 3 * i : VCOL + 3 * i + 3]

        cv2, dkx2, dky2 = pvec(0), pvec(1), pvec(2)
        iv2, mge2, mmod2 = pvec(3), pvec(4), pvec(5)
        mltn2, mmodn2 = pvec(6), pvec(7)

        mult = mybir.AluOpType.mult
        add = mybir.AluOpType.add

        @block.sync
        def _(sync):
            # outA rows 0..95: six 16-row pieces from the small tile,
            # chasing the vector-engine memset
            sync.wait_ge(vchain, 1)
            for i in range(6):
                dst = bass.AP(outA, i * 16 * M3, [[M3, 16], [1, M3]])
                sync.dma_start(dst, zts[:]).then_inc(zsemA, 16)
            # inv-dx broadcast rides the fill ring so it drains right after
            # the small pieces instead of crawling through the scalar ring
            sync.wait_ge(vchain, 11)
            with nc.allow_non_contiguous_dma(reason="partition broadcast"):
                sync.dma_start(ib[0:96, 0:1], rowA[0:1, :]).then_inc(bsem, 16)
            # the rest sources the gpsimd-memset big tile (128 partitions)
            sync.wait_ge(gz, 1)
            dst = bass.AP(outA, 96 * M3, [[M3, 128], [1, M3]])
            sync.dma_start(dst, ztb[:]).then_inc(zsemA, 16)
            dst = bass.AP(outA, 224 * M3, [[M3, 64], [1, M3]])
            sync.dma_start(dst, ztb[:, : M3 // 2]).then_inc(zsemA, 16)
            # outB block-row-1 rows 0..287: three 96-row pieces; the
            # constructed-tile strips ride the ring after the first piece
            dst = bass.AP(outB, 0, [[WB, 96], [1, WB]])
            sync.dma_start(dst, ztb[:, : M3 // 2]).then_inc(zsemB, 16)
            sync.wait_ge(vchain, NVOPS)
            sync.wait_ge(ctm, 1)
            with nc.allow_non_contiguous_dma(reason="diagonal strips"):
                for col0, base in ((J0, S6), (N + J0, S7), (N + J0 + 1, S8)):
                    dst = bass.AP(ct, col0, [[WB + 1, CTR], [1, 1]])
                    sync.dma_start(dst, vsl(base, J0 // 3, CTR // 3),
                                   single_packet=True).then_inc(strip_sem, 16)
            for i in range(1, 3):
                dst = bass.AP(outB, i * 96 * WB, [[WB, 96], [1, WB]])
                sync.dma_start(dst, ztb[:, : M3 // 2]).then_inc(zsemB, 16)
            # outB block-row-2 zero part: rows 288..449 (j = 0..161)
            dst = bass.AP(outB, 3 * 96 * WB, [[WB, 96], [1, WB]])
            sync.dma_start(dst, ztb[:, : M3 // 2]).then_inc(zsemC, 16)
            dst = bass.AP(outB, 4 * 96 * WB, [[WB, 66], [1, WB]])
            sync.dma_start(dst, ztb[:, : 66 * WB // 128]).then_inc(zsemC, 16)
            # constructed tile last: rows 450..575 with diagonals baked in
            sync.wait_ge(strip_sem, 48)
            dst = bass.AP(outB, (B + J0) * WB, [[WB, CTR], [1, WB]])
            sync.dma_start(dst, ct[0:CTR, :]).then_inc(ctd, 16)
            sync.wait_ge(ctd, 16)

        @block.gpsimd
        def _(g):
            g.memset(ztb[:], 0.0).then_inc(gz, 1)
            g.memset(ct[:], 0.0).then_inc(ctm, 1)

        @block.scalar
        def _(se):
            se.dma_start(pkb[:], pk[:]).then_inc(in_sem, 16)
            ndma = [0]

            def diag(tens, stride, col0, cnt, src, pair=False):
                dst = bass.AP(tens, col0,
                              [[stride, cnt], [1, 2 if pair else 1]])
                se.dma_start(dst, src, single_packet=True).then_inc(ddma, 16)
                ndma[0] += 1

            with nc.allow_non_contiguous_dma(reason="diagonal scatter"):
                se.wait_ge(vchain, NVOPS)
                # block row 0: full-width band on outA
                se.wait_ge(zsemA, 128)
                diag(outA, M3 + 1, 0, B, vsl(S0))
                diag(outA, M3 + 1, N - n, B, vsl(S1))
                diag(outA, M3 + 1, N, B, vsl(S2))
                diag(outA, M3 + 1, 2 * N - 1, B, vpair_src(P0), pair=True)
                # block row 1 on outB (rotated: A10 at 0/n, A11 at N)
                se.wait_ge(zsemB, 48)
                diag(outB, WB + 1, 0, B, vsl(S4))
                diag(outB, WB + 1, n, B, vsl(S5))
                diag(outB, WB + 1, N, B, vsl(S3))
                # block row 2 zero part: A22 single + A20 pair, j = 0..161
                se.wait_ge(zsemC, 32)
                b2 = B * WB
                diag(outB, WB + 1, b2, J0, vsl(S6, 0, J0 // 3))
                diag(outB, WB + 1, b2 + N, J0, vpair_src(P1, J0 // 3),
                     pair=True)
            assert ndma[0] == 9
            se.wait_ge(ddma, 16 * ndma[0])

        @block.vector
        def _(v):
            # no scoreboarding: dependent ops serialize through vchain with
            # producers/consumers interleaved >=2 apart so the semaphore
            # round-trip hides behind the interposed op
            cnt = [0]

            def op(ins, wait=None):
                cnt[0] += 1
                ins.then_inc(vchain, 1)
                if wait is not None:
                    v.wait_ge(vchain, wait)

            op(v.memset(zts[:], 0.0))                      # 1 small tile
            op(v.memset(vsl(S0), float(K0)))               # 2 A00 diag const
            op(v.memset(rowA[:], 1.0))                     # 3
            v.wait_ge(in_sem, 16)
            # global max(c): [32,72] reduce -> 32x32 block transpose -> max
            op(v.reduce_max(tsq[0:32, 0:1], pkb[0:32, CCOL:PKW],
                            axis=mybir.AxisListType.X), wait=4)
            op(v.transpose(tsqT[:], tsq[:]), wait=5)
            op(v.reduce_max(sc(0), tsqT[0:1, 0:32],
                            axis=mybir.AxisListType.X), wait=6)
            op(v.tensor_scalar_mul(sc(1), sc(0), float(DXC)), wait=7)
            op(v.reciprocal(sc(2), sc(1)))                 # 8
            op(v.tensor_mul(tslot(0), dkx2, iv2))          # 9  gx
            op(v.tensor_mul(tslot(4), dky2, iv2), wait=8)  # 10 gy
            op(v.tensor_scalar_mul(rowA[:], rowA[:], sc(2)), wait=9)  # 11
            op(v.tensor_scalar(tslot(1), tslot(0), 1.0, None, add), wait=10)
            op(v.tensor_scalar(tslot(5), tslot(4), 1.0, None, add), wait=12)
            op(v.reciprocal(tslot(2), tslot(1)), wait=13)  # 14 rxi
            op(v.reciprocal(tslot(6), tslot(5)))           # 15 ryi
            op(v.tensor_scalar(tslot(3), tslot(0), -1.0, 1.0, mult, add))
            op(v.tensor_scalar(tslot(7), tslot(4), -1.0, 1.0, mult, add),
               wait=16)
            op(v.tensor_mul(vsl(S3), tslot(3), tslot(2)), wait=17)   # A11
            op(v.tensor_mul(vsl(S6), tslot(7), tslot(6)))            # A22
            v.wait_ge(bsem, 16)
            op(v.tensor_scalar(tslot(9), cv2, ib[:, 0:1], float(MI),
                               mult, mult))                # 20 w
            op(v.tensor_scalar_mul(tslot(8), cv2, ib[:, 0:1]), wait=20)
            op(v.tensor_mul(vsl(S1), tslot(9), mge2), wait=21)       # w*mge
            op(v.tensor_mul(tslot(10), tslot(8), tslot(2)))          # 23 rx
            op(v.tensor_scalar_mul(vsl(S2), tslot(9), -1.0))         # -w
            op(v.tensor_mul(tslot(11), tslot(8), tslot(6)), wait=23)  # 25 ry
            op(v.tensor_mul(vstr(P0, 0), tslot(9), mmod2))           # w*mmod
            op(v.tensor_copy(vsl(S4), tslot(10)))                    # rx
            op(v.tensor_scalar_mul(vstr(P0, 1), tslot(9), -1.0))     # -w
            op(v.tensor_mul(vsl(S5), tslot(10), mltn2), wait=25)
            op(v.tensor_copy(vstr(P1, 0), tslot(11)))                # ry
            op(v.tensor_copy(vsl(S7), tslot(11)))                    # ry
            op(v.tensor_mul(vstr(P1, 1), tslot(11), mmodn2))
            op(v.tensor_mul(vsl(S8), tslot(11), mmodn2))
            assert cnt[0] == NVOPS, cnt[0]

    return nc


_nc_cache = None


def _get_nc() -> bass.Bass:
    global _nc_cache
    if _nc_cache is None:
        _nc_cache = _build_program()
    return _nc_cache


def _make_in_maps(c, dkx, dky):
    c = np.ascontiguousarray(c, dtype=np.float32)
    cT = np.ascontiguousarray(c.T).reshape(-1)
    dkxT = np.ascontiguousarray(np.asarray(dkx, np.float32).T).reshape(-1)
    dkyT = np.ascontiguousarray(np.asarray(dky, np.float32).T).reshape(-1)
    j = np.arange(N)
    iv = ((j // n) / 2.0).astype(np.float32)
    mge = (j >= n).astype(np.float32)
    mmod = (j % n != 0).astype(np.float32)
    mltn = np.where(j < N - n, -1.0, 0.0).astype(np.float32)
    mmodn = np.where((j + 1) % n != 0, -1.0, 0.0).astype(np.float32)

    in_maps = []
    for k in range(NCORES):
        sl = slice(k * B, (k + 1) * B)
        pkv = np.zeros((128, PKW), dtype=np.float32)
        pkv[0:32, CCOL:PKW] = c.reshape(32, 72)
        for i, vec in enumerate(
                [cT, dkxT, dkyT, iv, mge, mmod, mltn, mmodn]):
            pkv[0:96, VCOL + 3 * i : VCOL + 3 * i + 3] = vec[sl].reshape(96, 3)
        in_maps.append({"pk": pkv})
    return in_maps


def _assemble(resA, resB) -> np.ndarray:
    A = np.zeros((M3, M3), dtype=np.float32)
    for k in range(NCORES):
        off = k * B
        # device shards are bf16; widening to f32 is exact
        bandA = np.asarray(resA[k]).astype(np.float32)
        bandB = np.asarray(resB[k]).astype(np.float32)
        # block row 0: full width, rotated by off
        if off:
            A[off : off + B, off:] = bandA[:, : M3 - off]
            A[off : off + B, :off] = bandA[:, M3 - off :]
        else:
            A[:B, :] = bandA
        # block row 1: stored cols [0, WB) are global cols [off, off+WB)
        A[N + off : N + off + B, off : off + WB] = bandB[:B]
        # block row 2: stored cols map to global (2N + off + s) mod M3
        s0 = M3 - 2 * N - off
        A[2 * N + off : 2 * N + off + B, 2 * N + off :] = bandB[B:, :s0]
        A[2 * N + off : 2 * N + off + B, : WB - s0] = bandB[B:, s0:]
    return A


def kernel(c, dkx, dky, _trace=False):
    in_maps = _make_in_maps(c, dkx, dky)
    res = run_bass_kernel_spmd(
        _get_nc(), in_maps, core_ids=list(range(NCORES)), trace=_trace
    )
    A = _assemble([res.results[k]["outA"] for k in range(NCORES)],
                  [res.results[k]["outB"] for k in range(NCORES)])
    if _trace:
        return A, res
    return A


# revision 27
# speedup vs baseline: 1.0185x; 1.0185x over previous
"""Trainium2 Bass kernel for nn_EuESN_maml: assemble the 3N x 3N wave-equation
transition matrix A (N = 48*48) from c/dkx/dky fields.

A is all zeros except ~11 diagonals spread over the 9 N x N blocks, two of
which (A12/A21) are structurally zero. The kernel is a DMA memset of the
nonzero block bands plus strided diagonal scatter writes; the run is pure
HBM-write-bound (the fill runs at the ~358 GB/s per-core cap), so
  - the shard is stored as bfloat16 and widened to f32 on the host while
    gathering (<=2^-9 relative error, far inside the 2e-2 gate), and
  - each block row stores only its structurally nonzero band: block row 0
    is full width, block rows 1/2 are rotated so their diagonals land in
    stored columns [0, 2N) and the A12/A21 zero blocks are never written
    (the host zeros canvas provides them on unshard) - 22% fewer bytes.

Sharding (SPMD, 8 cores): block-row index partitioned. Core k owns rows
[288k, 288k+288) of each of the three N-row block rows of A. Per-band
column rotations make diagonal positions identical on every core (single
SPMD program); the host un-rotates with slice copies while gathering.

Engine plan per core:
  vector: small zero-tile memset, then the diagonal value vectors computed
         on 96 partitions ([96,3] row-major packing of each 288-vector)
         with the x/y chains interleaved to hide the no-scoreboard
         semaphore latency; the global max(c) finishes through one 32x32
         DVE block transpose; adjacent-diagonal pairs are written stride-2
         interleaved so one DMA covers both diagonals
  gpsimd: the big zero tile, then the constructed-tile memset
  sync  (HWDGE ring 0): contiguous zero-fill DMAs, all sourcing 128
         partitions, then the constructed tile: the whole block row 2
         band ([288, 4608] stored flat as [128, 10368]) with its three
         diagonals baked in, so nothing scatters after the fill ends
  scalar(HWDGE ring 1): input load, inv-dx broadcast (zero-stride
         replicated read), 2 constructed-tile strip writes, and 7
         diagonal scatter DMAs issued as soon as the zero pieces covering
         their rows have landed (fully inside the fill window)
"""

import math
import sys

import numpy as np

sys.path.insert(0, "/opt/trn_rl_repo")

import concourse.bass as bass
import concourse.mybir as mybir
from concourse.bass_utils import run_bass_kernel_spmd

# ---- problem constants (hardcoded from the nn_EuESN_maml spec) ----
n = 48
N = n * n            # 2304
M3 = 3 * N           # 6912 (output is M3 x M3)
NCORES = 8
B = N // NCORES      # 288 rows per sub-band
DT, CN, KP = 1.0, 0.1, 1e-4
MI = 1.0 / (1.0 / DT - KP / 2.0)          # 1/diagM (diagM is constant)
K0 = (1.0 / DT + KP / 2.0) * MI           # A00 diagonal value (constant)
DXC = (DT / CN) * math.sqrt(2.0)          # dx = DXC * max(c)

WB = 2 * N           # stored width of block rows 1/2 (4608)
CTR = 126            # constructed-tile rows: block-row-2 rows 162..287
J0 = B - CTR         # first sub-band-2-local row handled by the tile (162)

# value layout in vv [96, VW] bf16, value r of a slot at [r//3, base+r%3]:
# singles at 3*s; pairs P0/P1 stride-2 interleaved (two adjacent diagonals
# per 2-element descriptor)
S0, S1, S2, S3, S4, S5, S6 = 0, 3, 6, 9, 12, 15, 18
P0, P1 = 21, 27      # 6 cols each
S7, S8 = 33, 36      # plain copies of P1 halves for the strip sources
VW = 39

NVOPS = 33
# packed input: 8 value vectors as [96, 3] at cols 0..23, then the c grid
# as [32, 72] at cols 24..95 (32 partitions so one 32x32 block transpose
# finishes the global max)
VCOL = 0
CCOL = 24
PKW = CCOL + 72


def _build_program() -> bass.Bass:
    nc = bass.Bass()
    f32 = mybir.dt.float32
    bf16 = mybir.dt.bfloat16

    pk = nc.declare_dram_parameter("pk", [128, PKW], f32, isOutput=False)
    outA = nc.declare_dram_parameter("outA", [B, M3], bf16, isOutput=True)
    outB = nc.declare_dram_parameter("outB", [2 * B, WB], bf16, isOutput=True)

    from contextlib import ExitStack

    with ExitStack() as ctx:
        ec = ctx.enter_context
        zts = ec(nc.sbuf_tensor([128, 1728], bf16))    # small zero tile
        ztb = ec(nc.sbuf_tensor([128, M3], bf16))      # big zero tile
        ct = ec(nc.sbuf_tensor([128, WB], bf16))       # constructed tile
        pkb = ec(nc.sbuf_tensor([128, PKW], f32))      # packed inputs
        vv = ec(nc.sbuf_tensor([96, VW], bf16))        # diag value slots
        tmp = ec(nc.sbuf_tensor([96, 36], f32))        # [96,3] scratch x12
        tsq = ec(nc.sbuf_tensor([32, 32], f32))        # transpose in
        tsqT = ec(nc.sbuf_tensor([32, 32], f32))       # transpose out
        rowA = ec(nc.sbuf_tensor([1, 96], f32))        # inv-dx replicated
        ib = ec(nc.sbuf_tensor([96, 1], f32))          # inv-dx per partition
        scal = ec(nc.sbuf_tensor([1, 8], f32))         # scalars
        in_sem = ec(nc.semaphore("in_sem"))
        vchain = ec(nc.semaphore("vchain"))
        gz = ec(nc.semaphore("gz"))
        ctm = ec(nc.semaphore("ctm"))
        bsem = ec(nc.semaphore("bsem"))
        zsemA = ec(nc.semaphore("zsemA"))
        zsemB = ec(nc.semaphore("zsemB"))
        zsemC = ec(nc.semaphore("zsemC"))
        strip_sem = ec(nc.semaphore("strip_sem"))
        ddma = ec(nc.semaphore("ddma"))
        ctd = ec(nc.semaphore("ctd"))
        block = ec(nc.Block())

        def vsl(base, p0=0, np_=96, w=3):
            return vv[p0 : p0 + np_, base : base + w]

        def vstr(base, off):
            # stride-2 interleaved DVE write target inside a pair region
            return bass.AP(vv, base + off, [[VW, 96], [2, 3], [1, 1]])

        def vpair_src(base, np_=96):
            # pair DMA source: 2-element contiguous runs per value index
            return bass.AP(vv, base, [[VW, np_], [2, 3], [1, 2]])

        def tslot(s):
            return tmp[0:96, s * 3 : (s + 1) * 3]

        def sc(i):
            return scal[0:1, i : i + 1]

        def pvec(i):
            return pkb[0:96, VCOL + 3 * i : VCOL + 3 * i + 3]

        cv2, dkx2, dky2 = pvec(0), pvec(1), pvec(2)
        iv2, mge2, mmod2 = pvec(3), pvec(4), pvec(5)
        mltn2, mmodn2 = pvec(6), pvec(7)

        mult = mybir.AluOpType.mult
        add = mybir.AluOpType.add

        @block.sync
        def _(sync):
            # outA rows 0..95: three 32-row pieces from the small tile,
            # chasing the vector-engine memset
            sync.wait_ge(vchain, 1)
            for i in range(3):
                dst = bass.AP(outA, i * 32 * M3, [[M3, 32], [1, M3]])
                sync.dma_start(dst, zts[:]).then_inc(zsemA, 16)
            # inv-dx broadcast rides the fill ring so it drains right after
            # the small pieces instead of crawling through the scalar ring
            sync.wait_ge(vchain, 11)
            with nc.allow_non_contiguous_dma(reason="partition broadcast"):
                sync.dma_start(ib[0:96, 0:1], rowA[0:1, :]).then_inc(bsem, 16)
            # the rest sources the gpsimd-memset big tile (128 partitions)
            sync.wait_ge(gz, 1)
            dst = bass.AP(outA, 96 * M3, [[M3, 128], [1, M3]])
            sync.dma_start(dst, ztb[:]).then_inc(zsemA, 16)
            dst = bass.AP(outA, 224 * M3, [[M3, 64], [1, M3]])
            sync.dma_start(dst, ztb[:, : M3 // 2]).then_inc(zsemA, 16)
            # outB block-row-1 rows 0..287: three 96-row pieces; the
            # constructed-tile strips ride the ring after the first piece
            dst = bass.AP(outB, 0, [[WB, 96], [1, WB]])
            sync.dma_start(dst, ztb[:, : M3 // 2]).then_inc(zsemB, 16)
            sync.wait_ge(vchain, NVOPS)
            sync.wait_ge(ctm, 1)
            with nc.allow_non_contiguous_dma(reason="diagonal strips"):
                for col0, base in ((J0, S6), (N + J0, S7), (N + J0 + 1, S8)):
                    dst = bass.AP(ct, col0, [[WB + 1, CTR], [1, 1]])
                    sync.dma_start(dst, vsl(base, J0 // 3, CTR // 3),
                                   single_packet=True).then_inc(strip_sem, 16)
            for i in range(1, 3):
                dst = bass.AP(outB, i * 96 * WB, [[WB, 96], [1, WB]])
                sync.dma_start(dst, ztb[:, : M3 // 2]).then_inc(zsemB, 16)
            # outB block-row-2 zero part: rows 288..449 (j = 0..161)
            dst = bass.AP(outB, 3 * 96 * WB, [[WB, 96], [1, WB]])
            sync.dma_start(dst, ztb[:, : M3 // 2]).then_inc(zsemC, 16)
            dst = bass.AP(outB, 4 * 96 * WB, [[WB, 66], [1, WB]])
            sync.dma_start(dst, ztb[:, : 66 * WB // 128]).then_inc(zsemC, 16)
            # constructed tile last: rows 450..575 with diagonals baked in
            sync.wait_ge(strip_sem, 48)
            dst = bass.AP(outB, (B + J0) * WB, [[WB, CTR], [1, WB]])
            sync.dma_start(dst, ct[0:CTR, :]).then_inc(ctd, 16)
            sync.wait_ge(ctd, 16)

        @block.gpsimd
        def _(g):
            g.memset(ztb[:], 0.0).then_inc(gz, 1)
            g.memset(ct[:], 0.0).then_inc(ctm, 1)

        @block.scalar
        def _(se):
            se.dma_start(pkb[:], pk[:]).then_inc(in_sem, 16)
            ndma = [0]

            def diag(tens, stride, col0, cnt, src, pair=False):
                dst = bass.AP(tens, col0,
                              [[stride, cnt], [1, 2 if pair else 1]])
                se.dma_start(dst, src, single_packet=True).then_inc(ddma, 16)
                ndma[0] += 1

            with nc.allow_non_contiguous_dma(reason="diagonal scatter"):
                se.wait_ge(vchain, NVOPS)
                # block row 0: full-width band on outA
                se.wait_ge(zsemA, 80)
                diag(outA, M3 + 1, 0, B, vsl(S0))
                diag(outA, M3 + 1, N - n, B, vsl(S1))
                diag(outA, M3 + 1, N, B, vsl(S2))
                diag(outA, M3 + 1, 2 * N - 1, B, vpair_src(P0), pair=True)
                # block row 1 on outB (rotated: A10 at 0/n, A11 at N)
                se.wait_ge(zsemB, 48)
                diag(outB, WB + 1, 0, B, vsl(S4))
                diag(outB, WB + 1, n, B, vsl(S5))
                diag(outB, WB + 1, N, B, vsl(S3))
                # block row 2 zero part: A22 single + A20 pair, j = 0..161
                se.wait_ge(zsemC, 32)
                b2 = B * WB
                diag(outB, WB + 1, b2, J0, vsl(S6, 0, J0 // 3))
                diag(outB, WB + 1, b2 + N, J0, vpair_src(P1, J0 // 3),
                     pair=True)
            assert ndma[0] == 9
            se.wait_ge(ddma, 16 * ndma[0])

        @block.vector
        def _(v):
            # no scoreboarding: dependent ops serialize through vchain with
            # producers/consumers interleaved >=2 apart so the semaphore
            # round-trip hides behind the interposed op
            cnt = [0]

            def op(ins, wait=None):
                cnt[0] += 1
                ins.then_inc(vchain, 1)
                if wait is not None:
                    v.wait_ge(vchain, wait)

            op(v.memset(zts[:], 0.0))                      # 1 small tile
            op(v.memset(vsl(S0), float(K0)))               # 2 A00 diag const
            op(v.memset(rowA[:], 1.0))                     # 3
            v.wait_ge(in_sem, 16)
            # global max(c): [32,72] reduce -> 32x32 block transpose -> max
            op(v.reduce_max(tsq[0:32, 0:1], pkb[0:32, CCOL:PKW],
                            axis=mybir.AxisListType.X), wait=4)
            op(v.transpose(tsqT[:], tsq[:]), wait=5)
            op(v.reduce_max(sc(0), tsqT[0:1, 0:32],
                            axis=mybir.AxisListType.X), wait=6)
            op(v.tensor_scalar_mul(sc(1), sc(0), float(DXC)), wait=7)
            op(v.reciprocal(sc(2), sc(1)))                 # 8
            op(v.tensor_mul(tslot(0), dkx2, iv2))          # 9  gx
            op(v.tensor_mul(tslot(4), dky2, iv2), wait=8)  # 10 gy
            op(v.tensor_scalar_mul(rowA[:], rowA[:], sc(2)), wait=9)  # 11
            op(v.tensor_scalar(tslot(1), tslot(0), 1.0, None, add), wait=10)
            op(v.tensor_scalar(tslot(5), tslot(4), 1.0, None, add), wait=12)
            op(v.reciprocal(tslot(2), tslot(1)), wait=13)  # 14 rxi
            op(v.reciprocal(tslot(6), tslot(5)))           # 15 ryi
            op(v.tensor_scalar(tslot(3), tslot(0), -1.0, 1.0, mult, add))
            op(v.tensor_scalar(tslot(7), tslot(4), -1.0, 1.0, mult, add),
               wait=16)
            op(v.tensor_mul(vsl(S3), tslot(3), tslot(2)), wait=17)   # A11
            op(v.tensor_mul(vsl(S6), tslot(7), tslot(6)))            # A22
            v.wait_ge(bsem, 16)
            op(v.tensor_scalar(tslot(9), cv2, ib[:, 0:1], float(MI),
                               mult, mult))                # 20 w
            op(v.tensor_scalar_mul(tslot(8), cv2, ib[:, 0:1]), wait=20)
            op(v.tensor_mul(vsl(S1), tslot(9), mge2), wait=21)       # w*mge
            op(v.tensor_mul(tslot(10), tslot(8), tslot(2)))          # 23 rx
            op(v.tensor_scalar_mul(vsl(S2), tslot(9), -1.0))         # -w
            op(v.tensor_mul(tslot(11), tslot(8), tslot(6)), wait=23)  # 25 ry
            op(v.tensor_mul(vstr(P0, 0), tslot(9), mmod2))           # w*mmod
            op(v.tensor_copy(vsl(S4), tslot(10)))                    # rx
            op(v.tensor_scalar_mul(vstr(P0, 1), tslot(9), -1.0))     # -w
            op(v.tensor_mul(vsl(S5), tslot(10), mltn2), wait=25)
            op(v.tensor_copy(vstr(P1, 0), tslot(11)))                # ry
            op(v.tensor_copy(vsl(S7), tslot(11)))                    # ry
            op(v.tensor_mul(vstr(P1, 1), tslot(11), mmodn2))
            op(v.tensor_mul(vsl(S8), tslot(11), mmodn2))
            assert cnt[0] == NVOPS, cnt[0]

    return nc


_nc_cache = None


def _get_nc() -> bass.Bass:
    global _nc_cache
    if _nc_cache is None:
        _nc_cache = _build_program()
    return _nc_cache


def _make_in_maps(c, dkx, dky):
    c = np.ascontiguousarray(c, dtype=np.float32)
    cT = np.ascontiguousarray(c.T).reshape(-1)
    dkxT = np.ascontiguousarray(np.asarray(dkx, np.float32).T).reshape(-1)
    dkyT = np.ascontiguousarray(np.asarray(dky, np.float32).T).reshape(-1)
    j = np.arange(N)
    iv = ((j // n) / 2.0).astype(np.float32)
    mge = (j >= n).astype(np.float32)
    mmod = (j % n != 0).astype(np.float32)
    mltn = np.where(j < N - n, -1.0, 0.0).astype(np.float32)
    mmodn = np.where((j + 1) % n != 0, -1.0, 0.0).astype(np.float32)

    in_maps = []
    for k in range(NCORES):
        sl = slice(k * B, (k + 1) * B)
        pkv = np.zeros((128, PKW), dtype=np.float32)
        pkv[0:32, CCOL:PKW] = c.reshape(32, 72)
        for i, vec in enumerate(
                [cT, dkxT, dkyT, iv, mge, mmod, mltn, mmodn]):
            pkv[0:96, VCOL + 3 * i : VCOL + 3 * i + 3] = vec[sl].reshape(96, 3)
        in_maps.append({"pk": pkv})
    return in_maps


def _assemble(resA, resB) -> np.ndarray:
    A = np.zeros((M3, M3), dtype=np.float32)
    for k in range(NCORES):
        off = k * B
        # device shards are bf16; widening to f32 is exact
        bandA = np.asarray(resA[k]).astype(np.float32)
        bandB = np.asarray(resB[k]).astype(np.float32)
        # block row 0: full width, rotated by off
        if off:
            A[off : off + B, off:] = bandA[:, : M3 - off]
            A[off : off + B, :off] = bandA[:, M3 - off :]
        else:
            A[:B, :] = bandA
        # block row 1: stored cols [0, WB) are global cols [off, off+WB)
        A[N + off : N + off + B, off : off + WB] = bandB[:B]
        # block row 2: stored cols map to global (2N + off + s) mod M3
        s0 = M3 - 2 * N - off
        A[2 * N + off : 2 * N + off + B, 2 * N + off :] = bandB[B:, :s0]
        A[2 * N + off : 2 * N + off + B, : WB - s0] = bandB[B:, s0:]
    return A


def kernel(c, dkx, dky, _trace=False):
    in_maps = _make_in_maps(c, dkx, dky)
    res = run_bass_kernel_spmd(
        _get_nc(), in_maps, core_ids=list(range(NCORES)), trace=_trace
    )
    A = _assemble([res.results[k]["outA"] for k in range(NCORES)],
                  [res.results[k]["outB"] for k in range(NCORES)])
    if _trace:
        return A, res
    return A
